# revision 5
# baseline (speedup 1.0000x reference)
"""Multi-head attention (ScalableSoftmax) Trainium2 kernel, 8-core SPMD.

Problem: B=2, S=2048, D=1024, H=16 heads (HD=64).
  qkv = x @ qkv_w.T + qkv_b ; per head: softmax(0.2 * q k^T / 8) @ v
  out = vals @ out_w.T + out_b

Sharding: 2 heads per core (16 heads / 8 cores), Megatron-style:
  - qkv weights column-sharded by head -> each core computes q,k,v for
    its 2 heads over all B*S=4096 tokens (feature-major layout).
  - attention fully local per (batch, head).
  - out projection row-sharded: each core computes a partial [4096,1024]
    product over its 128 head-dims; host sums the 8 partials + out_b.

Device layout is feature-major ("transposed"): all big SBUF tensors are
[features(partitions), tokens(free)].  x is transposed on host once and
fed as xT so the contraction dim (d) lands on partitions.

All matmul operands use float32r (full-rate fp32 on the PE array).
"""
import numpy as np

import concourse.bass as bass
import concourse.mybir as mybir
import concourse.tile as tile
from concourse import bacc
from concourse.bass_utils import run_bass_kernel_spmd

B, S, D, H = 2, 2048, 1024, 16
HD = D // H          # 64
T = B * S            # 4096 tokens
N_CORES = 8
HPC = H // N_CORES   # heads per core = 2
SCALE = 0.2 / np.sqrt(float(HD))  # 0.025: folded softmax scale

F32R = mybir.dt.float32r
F32 = mybir.dt.float32
AF = mybir.ActivationFunctionType

N_SLICES = 8
SLICE = T // N_SLICES      # 512 tokens per QKV slice
N_TT = T // 128            # 32 token tiles of 128
SK_TILES = S // 128        # 16 key tiles per batch
SQ_TILES = S // 512        # 4 query tiles of 512 per batch
VW = 130                   # v_ext block: [ones|vA(64)] + [ones|vB(64)] per sk tile


def build_nc(reps: int = 1):
    nc = bacc.Bacc("TRN2")

    xT_d = nc.dram_tensor("xT", [128, 8, T], F32R, kind="ExternalInput")
    wT_d = nc.dram_tensor("wT", [128, 8, 3 * 128], F32R, kind="ExternalInput")
    bias_d = nc.dram_tensor("bias", [128, 2], F32, kind="ExternalInput")
    owT_d = nc.dram_tensor("owT", [128, D], F32R, kind="ExternalInput")
    idm_d = nc.dram_tensor("idm", [128, 128], F32R, kind="ExternalInput")
    cst_d = nc.dram_tensor("cst", [128, 192], F32R, kind="ExternalInput")
    out_d = nc.dram_tensor("out", [T, D], F32, kind="ExternalOutput")

    with tile.TileContext(nc) as tc:
        with (
            tc.tile_pool(name="persist", bufs=1) as persist,
            tc.tile_pool(name="xin", bufs=3) as xin,
            tc.tile_pool(name="stage", bufs=3) as stage,
            tc.tile_pool(name="expp", bufs=4) as expp,
            tc.tile_pool(name="normp", bufs=4) as normp,
            tc.tile_pool(name="outp", bufs=3) as outp,
            tc.tile_pool(name="psacc", bufs=2, space="PSUM") as psacc,
            tc.tile_pool(name="psgen", bufs=2, space="PSUM") as psgen,
        ):
            # ---- persistent tensors ----
            w_sb = persist.tile([128, 8, 3 * 128], F32R)
            nc.sync.dma_start(out=w_sb, in_=wT_d[:, :, :])
            owT_sb = persist.tile([128, D], F32R)
            nc.sync.dma_start(out=owT_sb, in_=owT_d[:, :])
            bias_sb = persist.tile([128, 2], F32)
            nc.sync.dma_start(out=bias_sb, in_=bias_d[:, :])
            idm_sb = persist.tile([128, 128], F32R)
            nc.sync.dma_start(out=idm_sb, in_=idm_d[:, :])

            q_sb = persist.tile([128, T], F32R)      # [qA;qB] feature-major
            k_sb = persist.tile([128, T], F32R)      # [kA;kB]
            v_sb = persist.tile([128, N_TT * VW], F32R)  # v_ext, natural layout
            valsT_sb = persist.tile([128, T], F32R)  # normalized attn output^T
            cst_sb = persist.tile([128, 192], F32R)
            nc.sync.dma_start(out=cst_sb, in_=cst_d[:, :])
            # ones columns of v_ext (cols g*130 and g*130+65) from host const
            v_ones_view = v_sb.rearrange("p (g c) -> p g c", c=VW)
            nc.sync.dma_start(
                out=v_ones_view[:, :, 0:1],
                in_=cst_d[:, 128:160].rearrange("p (g c) -> p g c", c=1),
            )
            nc.sync.dma_start(
                out=v_ones_view[:, :, 65:66],
                in_=cst_d[:, 160:192].rearrange("p (g c) -> p g c", c=1),
            )

            for rep in range(reps):
                # ================= QKV projection =================
                for ts in range(N_SLICES):
                    xts = xin.tile([128, 8, SLICE], F32R, tag="xts")
                    nc.sync.dma_start(
                        out=xts, in_=xT_d[:, :, ts * SLICE : (ts + 1) * SLICE]
                    )
                    col = slice(ts * SLICE, (ts + 1) * SLICE)
                    # j=0: q, j=1: k, j=2: v   (cols of wT per d-tile)
                    for j in range(3):
                        ps_qkv = psgen.tile([128, SLICE], F32, tag="sm", padded_shape=[128, 512])
                        for i in range(8):
                            nc.tensor.matmul(
                                ps_qkv,
                                w_sb[:, i, j * 128 : (j + 1) * 128],
                                xts[:, i, :],
                                start=(i == 0),
                                stop=(i == 7),
                            )
                        if j == 0:
                            nc.vector.tensor_scalar_add(
                                q_sb[:, col], ps_qkv, bias_sb[:, 0:1]
                            )
                        elif j == 1:
                            nc.vector.tensor_scalar_add(
                                k_sb[:, col], ps_qkv, bias_sb[:, 1:2]
                            )
                        else:
                            vts = stage.tile([128, SLICE], F32R, tag="vts")
                            nc.vector.tensor_copy(vts, ps_qkv)
                            # transpose v to natural layout, per 128-token tile
                            for u in range(SLICE // 128):
                                g = ts * 4 + u
                                pv_a = psgen.tile([128, 64], F32R, tag="sm", padded_shape=[128, 512])
                                nc.tensor.transpose(
                                    pv_a,
                                    vts[0:64, u * 128 : (u + 1) * 128],
                                    idm_sb[0:64, 0:64],
                                )
                                nc.vector.tensor_copy(
                                    v_sb[:, g * VW + 1 : g * VW + 65], pv_a
                                )
                                pv_b = psgen.tile([128, 64], F32R, tag="sm", padded_shape=[128, 512])
                                nc.tensor.transpose(
                                    pv_b,
                                    vts[64:128, u * 128 : (u + 1) * 128],
                                    idm_sb[64:128, 64:128],
                                )
                                nc.vector.tensor_copy(
                                    v_sb[:, g * VW + 66 : g * VW + 130], pv_b
                                )

                # v bias: natural layout -> bias varies along free dim; the
                # qkv_b v-part is folded on host into an extra xT column?  No:
                # handled by adding bv via the ones trick is not available.
                # Instead bv is folded on host into x's qkv product via an
                # appended constant input row (see host prep: x_aug).

                # ================= attention + norm + out-proj =================
                for b in range(B):
                    for sq in range(SQ_TILES):
                        qcol = slice(b * S + sq * 512, b * S + sq * 512 + 512)
                        ps_a = psacc.tile([65, 512], F32, tag="acc")
                        ps_b = psacc.tile([65, 512], F32, tag="acc")
                        for sk in range(SK_TILES):
                            g = b * SK_TILES + sk
                            kcol = slice(g * 128, (g + 1) * 128)
                            ps_s = psgen.tile([128, 1024], F32, tag="ps_s")
                            nc.tensor.matmul(
                                ps_s[:, 0:512],
                                k_sb[0:64, kcol],
                                q_sb[0:64, qcol],
                                start=True,
                                stop=True,
                                tile_position=(0, 0),
                            )
                            nc.tensor.matmul(
                                ps_s[:, 512:1024],
                                k_sb[64:128, kcol],
                                q_sb[64:128, qcol],
                                start=True,
                                stop=True,
                                tile_position=(64, 0),
                            )
                            exp_sb = expp.tile([128, 1024], F32R, tag="exp")
                            nc.scalar.activation(
                                exp_sb, ps_s, AF.Exp, scale=float(SCALE)
                            )
                            nc.tensor.matmul(
                                ps_a,
                                v_sb[:, g * VW : g * VW + 65],
                                exp_sb[:, 0:512],
                                start=(sk == 0),
                                stop=(sk == SK_TILES - 1),
                            )
                            nc.tensor.matmul(
                                ps_b,
                                v_sb[:, g * VW + 65 : g * VW + 130],
                                exp_sb[:, 512:1024],
                                start=(sk == 0),
                                stop=(sk == SK_TILES - 1),
                            )
                        # ---- normalize: vals / den, stack 2 heads ----
                        stag_a = normp.tile([65, 512], F32R, tag="stag")
                        stag_b = normp.tile([65, 512], F32R, tag="stag")
                        nc.vector.tensor_copy(stag_a, ps_a)
                        nc.vector.tensor_copy(stag_b, ps_b)
                        # assemble stacked raw vals + den rows via sb->sb DMA
                        comb = normp.tile([128, 512], F32R, tag="comb")
                        nc.sync.dma_start(out=comb[0:64, :], in_=stag_a[1:65, :])
                        nc.sync.dma_start(out=comb[64:128, :], in_=stag_b[1:65, :])
                        den2 = normp.tile([2, 512], F32R, tag="den2")
                        nc.sync.dma_start(out=den2[0:1, :], in_=stag_a[0:1, :])
                        nc.sync.dma_start(out=den2[1:2, :], in_=stag_b[0:1, :])
                        # broadcast den to matching partition halves (K=2 matmul)
                        den_ps = psgen.tile([128, 512], F32, tag="sm")
                        nc.tensor.matmul(
                            den_ps, cst_sb[0:2, 0:128], den2,
                            start=True, stop=True,
                        )
                        rec_sb = normp.tile([128, 512], F32R, tag="rec")
                        with nc.allow_low_precision("f32r reciprocal"):
                            nc.vector.reciprocal(rec_sb, den_ps)
                        nc.vector.tensor_mul(
                            valsT_sb[:, qcol], comb, rec_sb
                        )
                        # ---- out projection for these 4 token tiles ----
                        for u in range(4):
                            g = b * SK_TILES + sq * 4 + u
                            tcol = slice(g * 128, (g + 1) * 128)
                            o_sb = outp.tile([128, D], F32, tag="o_sb")
                            for n in range(2):
                                ps_o = psgen.tile([128, 512], F32, tag="sm")
                                nc.tensor.matmul(
                                    ps_o,
                                    valsT_sb[:, tcol],
                                    owT_sb[:, n * 512 : (n + 1) * 512],
                                    start=True,
                                    stop=True,
                                )
                                nc.vector.tensor_copy(
                                    o_sb[:, n * 512 : (n + 1) * 512], ps_o
                                )
                            nc.sync.dma_start(out=out_d[tcol, :], in_=o_sb)
    nc.compile()
    return nc


def _host_prep(x, qkv_w, qkv_b, out_w):
    """Build per-core input maps. Returns list of dicts."""
    xF = np.ascontiguousarray(x.reshape(T, D).astype(np.float32))
    # feature-major x with d split into 8 partition tiles: [128, 8, T]
    xT = np.ascontiguousarray(xF.T.reshape(8, 128, T).transpose(1, 0, 2))
    idm = np.eye(128, dtype=np.float32)
    cst = np.zeros((128, 192), dtype=np.float32)
    cst[0, 0:64] = 1.0      # sel2 row 0: denA -> out partitions 0-63
    cst[1, 64:128] = 1.0    # sel2 row 1: denB -> out partitions 64-127
    cst[:, 128:192] = 1.0   # ones source for v_ext ones columns
    in_maps = []
    for c in range(N_CORES):
        hA, hB = 2 * c, 2 * c + 1
        # per-head weight slices (rows of qkv_w): e = h*192 + [0:64 q |64:128 k|128:192 v]
        wq = np.concatenate(
            [qkv_w[hA * 192 : hA * 192 + 64], qkv_w[hB * 192 : hB * 192 + 64]], 0
        )  # [128, D]
        wk = np.concatenate(
            [
                qkv_w[hA * 192 + 64 : hA * 192 + 128],
                qkv_w[hB * 192 + 64 : hB * 192 + 128],
            ],
            0,
        )
        wv = np.concatenate(
            [
                qkv_w[hA * 192 + 128 : hA * 192 + 192],
                qkv_w[hB * 192 + 128 : hB * 192 + 192],
            ],
            0,
        )
        # wT: [D, 384] -> [128, 8, 384] (d on partitions)
        wT = np.concatenate([wq.T, wk.T, wv.T], axis=1)  # [D, 384]
        wT = np.ascontiguousarray(wT.reshape(8, 128, 384).transpose(1, 0, 2))
        bq = np.concatenate(
            [qkv_b[hA * 192 : hA * 192 + 64], qkv_b[hB * 192 : hB * 192 + 64]]
        )
        bk = np.concatenate(
            [
                qkv_b[hA * 192 + 64 : hA * 192 + 128],
                qkv_b[hB * 192 + 64 : hB * 192 + 128],
            ]
        )
        bias = np.stack([bq, bk], axis=1).astype(np.float32)  # [128, 2]
        # out_w columns for this core's head dims: d = h*64 + hd
        owT = np.concatenate(
            [
                out_w[:, hA * 64 : hA * 64 + 64].T,
                out_w[:, hB * 64 : hB * 64 + 64].T,
            ],
            0,
        )  # [128, D]
        in_maps.append(
            {
                "xT": xT,
                "wT": np.ascontiguousarray(wT, dtype=np.float32),
                "bias": bias,
                "owT": np.ascontiguousarray(owT, dtype=np.float32),
                "idm": idm,
                "cst": cst,
            }
        )
    return in_maps


_NC_CACHE = {}


def _get_nc(reps=1):
    if reps not in _NC_CACHE:
        _NC_CACHE[reps] = build_nc(reps)
    return _NC_CACHE[reps]


def kernel(x, qkv_w, qkv_b, out_w, out_b):
    x = np.asarray(x, dtype=np.float32)
    qkv_w = np.asarray(qkv_w, dtype=np.float32)
    qkv_b = np.asarray(qkv_b, dtype=np.float32)
    out_w = np.asarray(out_w, dtype=np.float32)
    out_b = np.asarray(out_b, dtype=np.float32)

    # NOTE: v bias handled here: v = x@wv.T + bv. The device kernel does NOT
    # add bv (free-dim broadcast is awkward); instead attention output using
    # v' = v - bv differs from truth by  sum_k attn[q,k] * bv = bv  (attn rows
    # sum to 1).  So vals = vals_dev + bv, and out = vals @ ow.T + out_b
    #   = vals_dev @ ow.T + (bv @ ow.T + out_b).
    # We fold bv @ out_w.T into the final host-side bias.
    bv = np.concatenate(
        [qkv_b[h * 192 + 128 : h * 192 + 192] for h in range(H)]
    )  # [D], ordered by head
    out_bias_eff = out_b + out_w @ bv  # out_w[e,d] * bv[d]

    in_maps = _host_prep(x, qkv_w, qkv_b, out_w)
    nc = _get_nc(reps=1)
    res = run_bass_kernel_spmd(nc, in_maps, core_ids=list(range(N_CORES)))
    acc = np.zeros((T, D), dtype=np.float64)
    for c in range(N_CORES):
        acc += res.results[c]["out"]
    acc += out_bias_eff
    return acc.reshape(B, S, D).astype(np.float32)


# revision 13
# speedup vs baseline: 1409.3324x; 1409.3324x over previous
"""Multi-head attention (ScalableSoftmax) Trainium2 kernel, 8-core SPMD.

Problem: B=2, S=2048, D=1024, H=16 heads (HD=64).
  qkv = x @ qkv_w.T + qkv_b ; per head: softmax(0.2 * q k^T / 8) @ v
  out = vals @ out_w.T + out_b

Sharding: 2 heads per core (16 heads / 8 cores), Megatron-style:
  - qkv weights column-sharded by head -> each core computes q,k,v for
    its 2 heads over all B*S=4096 tokens (feature-major layout).
  - attention fully local per (batch, head).
  - out projection row-sharded: each core computes a partial [4096,1024]
    product over its 128 head-dims; host sums the 8 partials + out_b.

Device layout is feature-major ("transposed"): all big SBUF tensors are
[features(partitions), tokens(free)].  x is transposed on host once and
fed as xT so the contraction dim (d) lands on partitions.

All matmul operands use float32r (full-rate fp32 on the PE array).
"""
import numpy as np

import concourse.bass as bass
import concourse.mybir as mybir
import concourse.tile as tile
from concourse import bacc
from concourse.bass_utils import run_bass_kernel_spmd

B, S, D, H = 2, 2048, 1024, 16
HD = D // H          # 64
T = B * S            # 4096 tokens
N_CORES = 8
HPC = H // N_CORES   # heads per core = 2
SCALE = 0.2 / np.sqrt(float(HD))  # 0.025: folded softmax scale

F32R = mybir.dt.float32r
F32 = mybir.dt.float32
AF = mybir.ActivationFunctionType

N_SLICES = 8
SLICE = T // N_SLICES      # 512 tokens per QKV slice
N_TT = T // 128            # 32 token tiles of 128
SK_TILES = S // 128        # 16 key tiles per batch
SQ_TILES = S // 512        # 4 query tiles of 512 per batch
VW = 130                   # v_ext block: [ones|vA(64)] + [ones|vB(64)] per sk tile


def build_nc(reps: int = 1):
    nc = bacc.Bacc("TRN2")

    xT_d = nc.dram_tensor("xT", [128, 8, T], F32R, kind="ExternalInput")
    wT_d = nc.dram_tensor("wT", [128, 8, 3 * 128], F32R, kind="ExternalInput")
    bias_d = nc.dram_tensor("bias", [128, 2], F32, kind="ExternalInput")
    owT_d = nc.dram_tensor("owT", [128, D], F32R, kind="ExternalInput")
    idm_d = nc.dram_tensor("idm", [128, 128], F32R, kind="ExternalInput")
    cst_d = nc.dram_tensor("cst", [128, 192], F32R, kind="ExternalInput")
    out_d = nc.dram_tensor("out", [T, D], F32, kind="ExternalOutput")

    with tile.TileContext(nc) as tc:
        with (
            tc.tile_pool(name="persist", bufs=1) as persist,
            tc.tile_pool(name="xin", bufs=3) as xin,
            tc.tile_pool(name="stage", bufs=3) as stage,
            tc.tile_pool(name="expp", bufs=6) as expp,
            tc.tile_pool(name="normp", bufs=4) as normp,
            tc.tile_pool(name="outp", bufs=3) as outp,
            tc.tile_pool(name="psacc", bufs=2, space="PSUM") as psacc,
            tc.tile_pool(name="psgen", bufs=2, space="PSUM") as psgen,
        ):
            # ---- persistent tensors ----
            w_sb = persist.tile([128, 8, 3 * 128], F32R)
            xts0 = xin.tile([128, 8, SLICE], F32R, tag="xts", name="xts0")
            nc.sync.dma_start(out=w_sb[:, 0, :], in_=wT_d[:, 0, :])
            nc.sync.dma_start(out=xts0[:, 0, :], in_=xT_d[:, 0, 0:SLICE])
            for i in range(1, 8):
                nc.sync.dma_start(out=w_sb[:, i, :], in_=wT_d[:, i, :])
            bias_sb = persist.tile([128, 2], F32)
            nc.sync.dma_start(out=bias_sb, in_=bias_d[:, :])
            idm_sb = persist.tile([128, 128], F32R)
            nc.sync.dma_start(out=idm_sb, in_=idm_d[:, :])
            owT_sb = persist.tile([128, D], F32R)
            cst_sb = persist.tile([128, 192], F32R)

            q_sb = persist.tile([128, T], F32R)      # [qA;qB] feature-major
            k_sb = persist.tile([128, T], F32R)      # [kA;kB]
            v_sb = persist.tile([128, N_TT * VW], F32R)  # v_ext, natural layout
            valsT_sb = persist.tile([128, T], F32R)  # normalized attn output^T

            # ones columns of v_ext (cols g*130 and g*130+65) from host const
            v_ones_view = v_sb.rearrange("p (g c) -> p g c", c=VW)
            nc.sync.dma_start(
                out=v_ones_view[:, :, 0:1],
                in_=cst_d[:, 128:160].rearrange("p (g c) -> p g c", c=1),
            )
            nc.sync.dma_start(
                out=v_ones_view[:, :, 65:66],
                in_=cst_d[:, 160:192].rearrange("p (g c) -> p g c", c=1),
            )

            def emit_qkv_unit(ts, j, xts):
                """One QKV unit: 8-step accumulation for feature group j of
                token slice ts, plus its PSUM->SBUF epilogue."""
                col = slice(ts * SLICE, (ts + 1) * SLICE)
                ps_qkv = psgen.tile(
                    [128, SLICE], F32, tag="sm", padded_shape=[128, 512],
                    name="ps_qkv",
                )
                for i in range(8):
                    nc.tensor.matmul(
                        ps_qkv,
                        w_sb[:, i, j * 128 : (j + 1) * 128],
                        xts[:, i, :],
                        start=(i == 0),
                        stop=(i == 7),
                    )
                if j == 0:
                    nc.vector.tensor_scalar_add(
                        q_sb[:, col], ps_qkv, bias_sb[:, 0:1]
                    )
                elif j == 1:
                    nc.vector.tensor_scalar_add(
                        k_sb[:, col], ps_qkv, bias_sb[:, 1:2]
                    )
                else:
                    vts = stage.tile([128, SLICE], F32R, tag="vts", name="vts")
                    nc.vector.tensor_copy(vts, ps_qkv)
                    # transpose v to natural [token, dim] layout per 128-tile
                    for u in range(SLICE // 128):
                        g = ts * 4 + u
                        pv_a = psgen.tile(
                            [128, 64], F32R, tag="sm",
                            padded_shape=[128, 512], name="pv_a",
                        )
                        nc.tensor.transpose(
                            pv_a,
                            vts[0:64, u * 128 : (u + 1) * 128],
                            idm_sb[0:64, 0:64],
                        )
                        nc.vector.tensor_copy(
                            v_sb[:, g * VW + 1 : g * VW + 65], pv_a
                        )
                        pv_b = psgen.tile(
                            [128, 64], F32R, tag="sm",
                            padded_shape=[128, 512], name="pv_b",
                        )
                        nc.tensor.transpose(
                            pv_b,
                            vts[64:128, u * 128 : (u + 1) * 128],
                            idm_sb[64:128, 64:128],
                        )
                        nc.vector.tensor_copy(
                            v_sb[:, g * VW + 66 : g * VW + 130], pv_b
                        )

            def qkv_half_units(half):
                """Return emission closures for this half's QKV work.
                One closure per (slice, feature-group); the slice DMA rides
                with its first unit."""
                units = []
                for ts in range(4 * half, 4 * half + 4):
                    xts_box = {}

                    def load(ts=ts, xts_box=xts_box):
                        if ts == 0:
                            for i in range(1, 8):
                                nc.sync.dma_start(
                                    out=xts0[:, i, :],
                                    in_=xT_d[:, i, 0:SLICE],
                                )
                            xts_box["t"] = xts0
                            return
                        xts = xin.tile([128, 8, SLICE], F32R, tag="xts",
                                       name="xts")
                        for i in range(8):
                            nc.sync.dma_start(
                                out=xts[:, i, :],
                                in_=xT_d[:, i, ts * SLICE : (ts + 1) * SLICE],
                            )
                        xts_box["t"] = xts

                    for j in range(3):
                        def unit(ts=ts, j=j, xts_box=xts_box, load=load):
                            if j == 0:
                                load()
                            emit_qkv_unit(ts, j, xts_box["t"])
                        units.append(unit)
                return units

            def outproj_units(b):
                """Deferrable out-projection closures for batch b (reads
                valsT_sb written by that batch's norm)."""
                units = []
                for g0 in range(b * SK_TILES, (b + 1) * SK_TILES):
                    def unit(g=g0):
                        tcol = slice(g * 128, (g + 1) * 128)
                        o_sb = outp.tile([128, D], F32, tag="o_sb",
                                         name="o_sb")
                        for n in range(2):
                            ps_o = psgen.tile([128, 512], F32, tag="sm",
                                              name="ps_o")
                            nc.tensor.matmul(
                                ps_o,
                                valsT_sb[:, tcol],
                                owT_sb[:, n * 512 : (n + 1) * 512],
                                start=True,
                                stop=True,
                            )
                            nc.any.tensor_copy(
                                o_sb[:, n * 512 : (n + 1) * 512], ps_o
                            )
                        nc.sync.dma_start(out=out_d[tcol, :], in_=o_sb)
                    units.append(unit)
                return units

            def attention_half(b, fillers, inline_out=False):
                """Attention + norm for batch b. Emits one filler unit after
                every few sk steps to keep PE fed while ACT runs the exps.
                inline_out: emit this batch's out-projection per sq group
                (used for the final half, which has no successor window)."""
                fill_iter = iter(fillers)
                n_steps = SQ_TILES * SK_TILES  # 64
                n_fill = len(fillers)
                emitted = 0
                step = 0
                for sq in range(SQ_TILES):
                    qcol = slice(b * S + sq * 512, b * S + sq * 512 + 512)
                    ps_a = psacc.tile([65, 512], F32, tag="acc", name="ps_a")
                    ps_b = psacc.tile([65, 512], F32, tag="acc", name="ps_b")
                    for sk in range(SK_TILES):
                        g = b * SK_TILES + sk
                        kcol = slice(g * 128, (g + 1) * 128)
                        ps_s = psgen.tile([128, 1024], F32, tag="ps_s",
                                          name="ps_s")
                        nc.tensor.matmul(
                            ps_s[:, 0:512],
                            k_sb[0:64, kcol],
                            q_sb[0:64, qcol],
                            start=True,
                            stop=True,
                            tile_position=(0, 0),
                        )
                        nc.tensor.matmul(
                            ps_s[:, 512:1024],
                            k_sb[64:128, kcol],
                            q_sb[64:128, qcol],
                            start=True,
                            stop=True,
                            tile_position=(64, 0),
                        )
                        exp_sb = expp.tile([128, 1024], F32R, tag="exp",
                                           name="exp_sb")
                        nc.scalar.activation(
                            exp_sb, ps_s, AF.Exp, scale=float(SCALE)
                        )
                        nc.tensor.matmul(
                            ps_a,
                            v_sb[:, g * VW : g * VW + 65],
                            exp_sb[:, 0:512],
                            start=(sk == 0),
                            stop=(sk == SK_TILES - 1),
                        )
                        nc.tensor.matmul(
                            ps_b,
                            v_sb[:, g * VW + 65 : g * VW + 130],
                            exp_sb[:, 512:1024],
                            start=(sk == 0),
                            stop=(sk == SK_TILES - 1),
                        )
                        step += 1
                        # evenly spread filler units across the 64 steps
                        want = (step * n_fill) // n_steps
                        while emitted < want:
                            next(fill_iter)()
                            emitted += 1
                    # ---- normalize: vals / den, stack 2 heads ----
                    stag_a = normp.tile([65, 512], F32R, tag="stag",
                                        name="stag_a")
                    stag_b = normp.tile([65, 512], F32R, tag="stag",
                                        name="stag_b")
                    nc.any.tensor_copy(stag_a, ps_a)
                    nc.any.tensor_copy(stag_b, ps_b)
                    # assemble stacked raw vals + den rows via sb->sb DMA
                    comb = normp.tile([128, 512], F32R, tag="comb",
                                      name="comb")
                    nc.sync.dma_start(out=comb[0:64, :], in_=stag_a[1:65, :])
                    nc.sync.dma_start(out=comb[64:128, :], in_=stag_b[1:65, :])
                    den2 = normp.tile([2, 512], F32R, tag="den2", name="den2")
                    nc.sync.dma_start(out=den2[0:1, :], in_=stag_a[0:1, :])
                    nc.sync.dma_start(out=den2[1:2, :], in_=stag_b[0:1, :])
                    # broadcast den to matching partition halves (K=2 matmul)
                    den_ps = psgen.tile([128, 512], F32, tag="sm",
                                        name="den_ps")
                    nc.tensor.matmul(
                        den_ps, cst_sb[0:2, 0:128], den2,
                        start=True, stop=True,
                    )
                    rec_sb = normp.tile([128, 512], F32R, tag="rec",
                                        name="rec_sb")
                    with nc.allow_low_precision("f32r reciprocal"):
                        nc.vector.reciprocal(rec_sb, den_ps)
                    nc.vector.tensor_mul(valsT_sb[:, qcol], comb, rec_sb)
                    if inline_out:
                        for u_ in outproj_units(b)[sq * 4 : sq * 4 + 4]:
                            u_()

            # software pipeline over (rep, half): the next half's QKV and
            # the previous half's out-projection are interleaved into the
            # current half's (ACT-bound) attention steps.
            for u in qkv_half_units(0):
                u()
            nc.sync.dma_start(out=owT_sb, in_=owT_d[:, :])
            nc.sync.dma_start(out=cst_sb, in_=cst_d[:, :])
            halves = [(r, h) for r in range(reps) for h in range(2)]
            pending_out = []
            for idx, (r, h) in enumerate(halves):
                fillers = list(pending_out)
                if idx + 1 < len(halves):
                    qu = qkv_half_units(halves[idx + 1][1])
                    # alternate qkv and outproj units for even engine load
                    merged = []
                    qi, oi = 0, 0
                    while qi < len(qu) or oi < len(fillers):
                        if qi < len(qu):
                            merged.append(qu[qi]); qi += 1
                        if oi < len(fillers):
                            merged.append(fillers[oi]); oi += 1
                    fillers = merged
                last = idx + 1 == len(halves)
                attention_half(h, fillers, inline_out=last)
                pending_out = [] if last else outproj_units(h)
    nc.compile()
    return nc


def _host_prep(x, qkv_w, qkv_b, out_w):
    """Build per-core input maps. Returns list of dicts."""
    xF = np.ascontiguousarray(x.reshape(T, D).astype(np.float32))
    # feature-major x with d split into 8 partition tiles: [128, 8, T]
    xT = np.ascontiguousarray(xF.T.reshape(8, 128, T).transpose(1, 0, 2))
    idm = np.eye(128, dtype=np.float32)
    cst = np.zeros((128, 192), dtype=np.float32)
    cst[0, 0:64] = 1.0      # sel2 row 0: denA -> out partitions 0-63
    cst[1, 64:128] = 1.0    # sel2 row 1: denB -> out partitions 64-127
    cst[:, 128:192] = 1.0   # ones source for v_ext ones columns
    in_maps = []
    for c in range(N_CORES):
        hA, hB = 2 * c, 2 * c + 1
        # per-head weight slices (rows of qkv_w): e = h*192 + [0:64 q |64:128 k|128:192 v]
        wq = np.concatenate(
            [qkv_w[hA * 192 : hA * 192 + 64], qkv_w[hB * 192 : hB * 192 + 64]], 0
        )  # [128, D]
        wk = np.concatenate(
            [
                qkv_w[hA * 192 + 64 : hA * 192 + 128],
                qkv_w[hB * 192 + 64 : hB * 192 + 128],
            ],
            0,
        )
        wv = np.concatenate(
            [
                qkv_w[hA * 192 + 128 : hA * 192 + 192],
                qkv_w[hB * 192 + 128 : hB * 192 + 192],
            ],
            0,
        )
        # wT: [D, 384] -> [128, 8, 384] (d on partitions)
        wT = np.concatenate([wq.T, wk.T, wv.T], axis=1)  # [D, 384]
        wT = np.ascontiguousarray(wT.reshape(8, 128, 384).transpose(1, 0, 2))
        bq = np.concatenate(
            [qkv_b[hA * 192 : hA * 192 + 64], qkv_b[hB * 192 : hB * 192 + 64]]
        )
        bk = np.concatenate(
            [
                qkv_b[hA * 192 + 64 : hA * 192 + 128],
                qkv_b[hB * 192 + 64 : hB * 192 + 128],
            ]
        )
        bias = np.stack([bq, bk], axis=1).astype(np.float32)  # [128, 2]
        # out_w columns for this core's head dims: d = h*64 + hd
        owT = np.concatenate(
            [
                out_w[:, hA * 64 : hA * 64 + 64].T,
                out_w[:, hB * 64 : hB * 64 + 64].T,
            ],
            0,
        )  # [128, D]
        in_maps.append(
            {
                "xT": xT,
                "wT": np.ascontiguousarray(wT, dtype=np.float32),
                "bias": bias,
                "owT": np.ascontiguousarray(owT, dtype=np.float32),
                "idm": idm,
                "cst": cst,
            }
        )
    return in_maps


_NC_CACHE = {}


def _get_nc(reps=1):
    if reps not in _NC_CACHE:
        _NC_CACHE[reps] = build_nc(reps)
    return _NC_CACHE[reps]


def kernel(x, qkv_w, qkv_b, out_w, out_b):
    x = np.asarray(x, dtype=np.float32)
    qkv_w = np.asarray(qkv_w, dtype=np.float32)
    qkv_b = np.asarray(qkv_b, dtype=np.float32)
    out_w = np.asarray(out_w, dtype=np.float32)
    out_b = np.asarray(out_b, dtype=np.float32)

    # NOTE: v bias handled here: v = x@wv.T + bv. The device kernel does NOT
    # add bv (free-dim broadcast is awkward); instead attention output using
    # v' = v - bv differs from truth by  sum_k attn[q,k] * bv = bv  (attn rows
    # sum to 1).  So vals = vals_dev + bv, and out = vals @ ow.T + out_b
    #   = vals_dev @ ow.T + (bv @ ow.T + out_b).
    # We fold bv @ out_w.T into the final host-side bias.
    bv = np.concatenate(
        [qkv_b[h * 192 + 128 : h * 192 + 192] for h in range(H)]
    )  # [D], ordered by head
    out_bias_eff = out_b + out_w @ bv  # out_w[e,d] * bv[d]

    in_maps = _host_prep(x, qkv_w, qkv_b, out_w)
    nc = _get_nc(reps=1)
    res = run_bass_kernel_spmd(nc, in_maps, core_ids=list(range(N_CORES)))
    acc = np.zeros((T, D), dtype=np.float64)
    for c in range(N_CORES):
        acc += res.results[c]["out"]
    acc += out_bias_eff
    return acc.reshape(B, S, D).astype(np.float32)
